# revision 17
# baseline (speedup 1.0000x reference)
"""BSGRUv2 Trainium2 Bass kernel.

Strategy: batch dim B=64 sharded over 8 NeuronCores (B_local=8); weights
replicated.  Everything in the recurrence lives transposed (H / 3H on SBUF
partitions, batch on the free dim):

  pre-pass (per core):
    xT  [I, T*BL]  (DMA transpose)
    Wx  = W_w @ x + (W_b + [U_b_z, U_b_r, 0])  via PE (f32r), staged to DRAM
          in layout [t, m, b, p]  (m = 3H/128 chunk, p = partition)
  recurrence (t = 0..T-1), per step:
    scores[b,n] = h@K_h.T + x_t@K_wI.T + K_b      (7 small accumulating MMs)
    ky = exp(tau*scores) / sum                     (ACT exp + DVE)
    ky_bc[128,4,BL] = E_j @ ky.T                   (4 small MMs, bf16)
    P[128,12,BL] = U.T-tiles @ v-tiles             (48 bf16 MMs, fp32 PSUM)
    gates: z,r = 0.5*tanh(0.5*(P+Wx)*ky)+0.5 ; n = tanh(((P+Ub)*r+Wx)*ky)
    h' = n + z*(h-n) ; v' = h'*ky (bf16)
"""
import sys

if "/opt/trn_rl_repo" not in sys.path:
    sys.path.insert(0, "/opt/trn_rl_repo")

import numpy as np

T, B, I, H, NB = 512, 64, 256, 512, 8
NCORES = 8
BL = B // NCORES          # 8
M3H = 3 * H // 128        # 12
KJ = H // 128             # 4
KI = I // 128             # 2
TAU = 5.0

_CACHE: dict = {}


def _build(Tsteps: int, u_dtype: str = "bf16"):
    import concourse.bass as bass
    import concourse.bacc as bacc
    import concourse.tile as tile
    from concourse import mybir
    from contextlib import ExitStack

    fp32 = mybir.dt.float32
    bf16 = mybir.dt.bfloat16
    f32r = mybir.dt.float32r
    AF = mybir.ActivationFunctionType
    ALU = mybir.AluOpType
    AX = mybir.AxisListType

    udt = bf16 if u_dtype == "bf16" else fp32

    nc = bacc.Bacc("TRN2", target_bir_lowering=False, debug=False,
                   enable_asserts=False)

    x = nc.dram_tensor("x", [Tsteps, BL, I], fp32, kind="ExternalInput").ap()
    K_w = nc.dram_tensor("K_w", [NB, I + H], fp32, kind="ExternalInput").ap()
    K_b = nc.dram_tensor("K_b", [NB], fp32, kind="ExternalInput").ap()
    W_w = nc.dram_tensor("W_w", [3 * H, I], fp32, kind="ExternalInput").ap()
    W_b = nc.dram_tensor("W_b", [3 * H], fp32, kind="ExternalInput").ap()
    U_w = nc.dram_tensor("U_w", [3 * H, H], fp32, kind="ExternalInput").ap()
    U_b = nc.dram_tensor("U_b", [3 * H], fp32, kind="ExternalInput").ap()
    out_seq = nc.dram_tensor("out_seq", [Tsteps, BL, H], fp32,
                             kind="ExternalOutput").ap()
    out_blk = nc.dram_tensor("out_blk", [Tsteps, BL, NB], fp32,
                             kind="ExternalOutput").ap()

    TBL = Tsteps * BL

    with tile.TileContext(nc) as tc:
        with ExitStack() as ctx:
            singles = ctx.enter_context(tc.tile_pool(name="singles", bufs=1))
            dram = ctx.enter_context(
                tc.tile_pool(name="dram", bufs=1, space="DRAM"))
            p_wx = ctx.enter_context(
                tc.tile_pool(name="p_wx", bufs=2, space="PSUM"))
            sb_wx = ctx.enter_context(tc.tile_pool(name="sb_wx", bufs=3))
            # recurrence pools
            p_P = ctx.enter_context(
                tc.tile_pool(name="p_P", bufs=2, space="PSUM"))
            p_sc = ctx.enter_context(
                tc.tile_pool(name="p_sc", bufs=2, space="PSUM"))
            p_ky = ctx.enter_context(
                tc.tile_pool(name="p_ky", bufs=2, space="PSUM"))
            wx_ld = ctx.enter_context(tc.tile_pool(name="wx_ld", bufs=3))
            gates = ctx.enter_context(tc.tile_pool(name="gates", bufs=3))
            hv = ctx.enter_context(tc.tile_pool(name="hv", bufs=3))
            small = ctx.enter_context(tc.tile_pool(name="small", bufs=4))

            # ---------------- pre-pass: constants into SBUF ----------------
            xt = singles.tile([128, KI, TBL], fp32)
            xflat = x.rearrange("t b i -> (t b) i")
            NSPLIT = 8
            for k in range(KI):
                for s in range(NSPLIT):
                    seg = TBL // NSPLIT
                    nc.sync.dma_start(
                        out=xt[:, k, s * seg:(s + 1) * seg],
                        in_=xflat[s * seg:(s + 1) * seg,
                                  k * 128:(k + 1) * 128].rearrange("n p -> p n"))

            wwt = singles.tile([128, KI, 3 * H], fp32)
            for k in range(KI):
                for s in range(4):
                    seg = 3 * H // 4
                    nc.sync.dma_start(
                        out=wwt[:, k, s * seg:(s + 1) * seg],
                        in_=W_w[s * seg:(s + 1) * seg,
                                k * 128:(k + 1) * 128].rearrange("m p -> p m"))


            uwt32 = singles.tile([128, KJ, 3 * H], fp32)
            for k in range(KJ):
                for s in range(4):
                    seg = 3 * H // 4
                    nc.sync.dma_start(
                        out=uwt32[:, k, s * seg:(s + 1) * seg],
                        in_=U_w[s * seg:(s + 1) * seg,
                                k * 128:(k + 1) * 128].rearrange("m p -> p m"))
            if udt == bf16:
                ubf = singles.tile([128, KJ, 3 * H], bf16)
                for k in range(KJ):
                    nc.vector.tensor_copy(out=ubf[:, k, :], in_=uwt32[:, k, :])
            else:
                ubf = uwt32

            kht = singles.tile([128, KJ, NB], fp32)
            for j in range(KJ):
                nc.sync.dma_start(
                    out=kht[:, j, :],
                    in_=K_w[:, I + j * 128:I + (j + 1) * 128]
                    .rearrange("n p -> p n"))
            kit = singles.tile([128, KI, NB], fp32)
            for k in range(KI):
                nc.sync.dma_start(
                    out=kit[:, k, :],
                    in_=K_w[:, k * 128:(k + 1) * 128].rearrange("n p -> p n"))
            kbr = singles.tile([1, NB], fp32)
            nc.sync.dma_start(out=kbr, in_=K_b.unsqueeze(0))
            ones1 = singles.tile([1, NB], fp32)
            nc.vector.memset(ones1, 1.0)

            ebf = singles.tile([NB, KJ, 128], bf16)
            ones64 = singles.tile([1, 64], bf16)
            nc.vector.memset(ebf, 0.0)
            nc.vector.memset(ones64, 1.0)
            for j in range(KJ):
                for half in range(2):
                    nbi = 2 * j + half
                    nc.sync.dma_start(
                        out=ebf[nbi:nbi + 1, j, half * 64:(half + 1) * 64],
                        in_=ones64)

            ubncol = singles.tile([128, KJ], fp32)
            nc.sync.dma_start(
                out=ubncol, in_=U_b[2 * H:].rearrange("(m p) -> p m", p=128))
            ubn = ubncol.unsqueeze(2).broadcast_to([128, KJ, BL])

            wbias = singles.tile([128, M3H], fp32)
            ubtmp = singles.tile([128, M3H], fp32)
            nc.sync.dma_start(
                out=wbias, in_=W_b.rearrange("(m p) -> p m", p=128))
            nc.sync.dma_start(
                out=ubtmp, in_=U_b.rearrange("(m p) -> p m", p=128))
            nc.vector.memset(ubtmp[:, 2 * KJ:3 * KJ], 0.0)
            nc.vector.tensor_add(out=wbias, in0=wbias, in1=ubtmp)

            # ---------------- pre-pass: Wx GEMM -> DRAM ----------------
            wx_d = dram.tile([M3H, 128, TBL], fp32)
            CH = min(512, TBL)
            NCH = TBL // CH
            for nch in range(NCH):
                for m in range(M3H):
                    pw = p_wx.tile([128, CH], fp32)
                    for k in range(KI):
                        nc.tensor.matmul(
                            pw,
                            wwt[:, k, m * 128:(m + 1) * 128],
                            xt[:, k, nch * CH:(nch + 1) * CH],
                            start=(k == 0), stop=(k == KI - 1))
                    ws = sb_wx.tile([128, CH], fp32)
                    nc.scalar.activation(out=ws, in_=pw, func=AF.Identity,
                                         bias=wbias[:, m:m + 1], scale=1.0)
                    nc.sync.dma_start(
                        out=wx_d[m, :, nch * CH:(nch + 1) * CH], in_=ws)

            # ---------------- recurrence state ----------------
            h_t = hv.tile([128, KJ, BL], fp32, tag="h")
            v_t = hv.tile([128, KJ, BL], udt, tag="v")
            nc.vector.memset(h_t, 0.0)
            nc.vector.memset(v_t, 0.0)

            ky32 = [singles.tile([32, 32], fp32, name=f"ky32_{i}", tag=f"ky32_{i}")
                    for i in range(2)]
            kyT32 = [singles.tile([32, 32], fp32, name=f"kyT32_{i}", tag=f"kyT32_{i}")
                     for i in range(2)]
            for tl in ky32 + kyT32:
                nc.vector.memset(tl, 0.0)

            blk_acc = singles.tile([BL, Tsteps * NB], fp32)

            for t in range(Tsteps):
                # Wx slab for this step
                wx_t = wx_ld.tile([128, M3H, BL], fp32, tag="wx")
                nc.sync.dma_start(
                    out=wx_t,
                    in_=wx_d[:, :, t * BL:(t + 1) * BL].transpose([1, 0, 2]))

                # --- scores = h@K_h.T + x_t@K_wI.T + K_b  -> PSUM [BL, NB]
                sc = p_sc.tile([BL, NB], fp32, tag="sc")
                for k in range(KI):
                    nc.tensor.matmul(sc, xt[:, k, t * BL:(t + 1) * BL], kit[:, k, :],
                                     start=(k == 0), stop=False)
                nc.tensor.matmul(sc, ones1, kbr, start=False, stop=False)
                for j in range(KJ):
                    nc.tensor.matmul(sc, h_t[:, j, :], kht[:, j, :],
                                     start=False, stop=(j == KJ - 1))

                # --- softmax over NB (free dim), tau folded into exp scale
                ex = small.tile([BL, NB], fp32, tag="ex")
                nc.scalar.activation(out=ex, in_=sc, func=AF.Exp, scale=TAU)
                s8 = small.tile([BL, 1], fp32, tag="s8")
                nc.vector.reduce_sum(out=s8, in_=ex, axis=AX.X)
                r8 = small.tile([BL, 1], fp32, tag="r8")
                nc.vector.reciprocal(out=r8, in_=s8)
                kyt = ky32[t % 2]
                nc.vector.tensor_scalar_mul(out=kyt[:BL, :NB], in0=ex, scalar1=r8)
                nc.vector.tensor_copy(
                    out=blk_acc[:, t * NB:(t + 1) * NB], in_=kyt[:BL, :NB])
                kyTt = kyT32[t % 2]
                nc.vector.transpose(out=kyTt, in_=kyt)
                kyT_bf = small.tile([NB, BL], udt, tag="kyT")
                nc.vector.tensor_copy(out=kyT_bf, in_=kyTt[:NB, :BL])

                # --- P = U @ v (one PSUM accumulation group, z,r chunks first)
                P = p_P.tile([128, M3H, BL], fp32, tag="P")
                for m in range(2 * KJ):
                    for k in range(KJ):
                        nc.tensor.matmul(
                            P[:, m, :], ubf[:, k, m * 128:(m + 1) * 128],
                            v_t[:, k, :],
                            start=(m == 0 and k == 0), stop=False)

                # --- ky broadcast: ky_bc[:, j, :] = E_j.T @ ky.T
                kybc = p_ky.tile([128, KJ, BL], fp32, tag="kybc")
                for j in range(KJ):
                    nc.tensor.matmul(kybc[:, j, :], ebf[:, j, :], kyT_bf,
                                     start=(j == 0), stop=(j == KJ - 1))

                for m in range(2 * KJ, M3H):
                    for k in range(KJ):
                        nc.tensor.matmul(
                            P[:, m, :], ubf[:, k, m * 128:(m + 1) * 128],
                            v_t[:, k, :],
                            start=False, stop=(m == M3H - 1 and k == KJ - 1))

                # --- z, r gates
                zr = gates.tile([128, 2 * KJ, BL], fp32, tag="zr")
                nc.vector.tensor_add(out=zr, in0=P[:, :2 * KJ, :],
                                     in1=wx_t[:, :2 * KJ, :])
                nc.vector.tensor_mul(out=zr[:, :KJ, :], in0=zr[:, :KJ, :],
                                     in1=kybc)
                nc.vector.tensor_mul(out=zr[:, KJ:, :], in0=zr[:, KJ:, :],
                                     in1=kybc)
                nc.scalar.activation(out=zr, in_=zr, func=AF.Tanh, scale=0.5)
                nc.vector.tensor_scalar(out=zr, in0=zr, scalar1=0.5,
                                        scalar2=0.5, op0=ALU.mult, op1=ALU.add)

                # --- n gate
                an = gates.tile([128, KJ, BL], fp32, tag="an")
                nc.vector.tensor_add(out=an, in0=P[:, 2 * KJ:, :], in1=ubn)
                nc.vector.tensor_mul(out=an, in0=an, in1=zr[:, KJ:, :])
                nc.vector.tensor_add(out=an, in0=an, in1=wx_t[:, 2 * KJ:, :])
                nc.vector.tensor_mul(out=an, in0=an, in1=kybc)
                nt = gates.tile([128, KJ, BL], fp32, tag="nt")
                nc.scalar.activation(out=nt, in_=an, func=AF.Tanh)

                # --- h' = n + z*(h-n);  v' = h'*ky (cast to udt)
                dt_ = gates.tile([128, KJ, BL], fp32, tag="dt")
                nc.vector.tensor_sub(out=dt_, in0=h_t, in1=nt)
                nc.vector.tensor_mul(out=dt_, in0=dt_, in1=zr[:, :KJ, :])
                h_new = hv.tile([128, KJ, BL], fp32, tag="h")
                nc.vector.tensor_add(out=h_new, in0=dt_, in1=nt)
                v_new = hv.tile([128, KJ, BL], udt, tag="v")
                nc.vector.tensor_mul(out=v_new, in0=h_new, in1=kybc)

                for j in range(KJ):
                    nc.sync.dma_start(
                        out=out_seq[t, :, j * 128:(j + 1) * 128]
                        .rearrange("b p -> p b"),
                        in_=h_new[:, j, :])

                h_t, v_t = h_new, v_new

            nc.sync.dma_start(
                out=out_blk.transpose([1, 0, 2]),
                in_=blk_acc.rearrange("b (t n) -> b t n", n=NB))

    nc.compile()
    return nc


def _get_nc(Tsteps: int, u_dtype: str = "bf16"):
    key = (Tsteps, u_dtype)
    if key not in _CACHE:
        _CACHE[key] = _build(Tsteps, u_dtype)
    return _CACHE[key]


def kernel(in_tensor, K_w, K_b, W_w, W_b, U_w, U_b, _trace=False):
    from concourse import bass_utils

    nc = _get_nc(T)
    shared = {
        "K_w": np.ascontiguousarray(K_w, np.float32),
        "K_b": np.ascontiguousarray(K_b, np.float32),
        "W_w": np.ascontiguousarray(W_w, np.float32),
        "W_b": np.ascontiguousarray(W_b, np.float32),
        "U_w": np.ascontiguousarray(U_w, np.float32),
        "U_b": np.ascontiguousarray(U_b, np.float32),
    }
    in_maps = []
    for c in range(NCORES):
        m = dict(shared)
        m["x"] = np.ascontiguousarray(
            in_tensor[:, c * BL:(c + 1) * BL, :], np.float32)
        in_maps.append(m)
    res = bass_utils.run_bass_kernel_spmd(
        nc, in_maps, core_ids=list(range(NCORES)), trace=_trace)
    out_seq = np.concatenate([res.results[c]["out_seq"] for c in range(NCORES)],
                             axis=1)
    out_blk = np.concatenate([res.results[c]["out_blk"] for c in range(NCORES)],
                             axis=1)
    kernel._last_result = res
    return out_seq, out_blk
